# revision 10
# baseline (speedup 1.0000x reference)
"""GRU cell kernel for Trainium2, data-parallel across 8 NeuronCores.

Per core: batch shard of 1024 rows; weights replicated.
  u  = sigmoid(x @ Wxu + h @ Whu + bu)
  r  = sigmoid(x @ Wxr + h @ Whr + br)
  c' = tanh  (x @ Wxc + (h*r) @ Whc + bc)
  c  = u*c' + (1-u)*h

Restructured from the baseline for dense PE occupancy:
  - all activations kept transposed [feature, batch] in bf16; weights cast
    to bf16 on DVE; matmuls bf16 with fp32 PSUM accumulation
  - DMA emission order matches compute-consumption order so gate r's
    matmuls start ~7us in and are paced by weight-chunk arrival
  - gate r batch-half 0 runs "k-outer" over a 6-wide j-group (6 PSUM
    banks) so each arriving weight chunk feeds 6 matmuls immediately
  - PE transposes write bf16 PSUM batches of 8; single strided DVE copy
    per batch
  - output is stored as bf16 (host upcasts); biases ride the gpsimd ring
"""

import os
import sys

import numpy as np

B = 8192
E = 1024
H = 1024
NCORES = 8
B_SH = B // NCORES  # 1024 rows per core

P = 128
KE = E // P   # 8 contraction chunks per operand side
NJ = H // P   # 8 output feature chunks
BN = 512      # moving free-dim per matmul
NB = B_SH // BN  # 2

W_NAMES = ("Wxu", "Whu", "Wxr", "Whr", "Wxc", "Whc")
B_NAMES = ("bu", "br", "bc")

_NC_CACHE = {}


def _ensure_paths():
    for p in ("/opt/trn_rl_repo", "/root/.axon_site/_ro/trn_rl_repo"):
        if os.path.isdir(p) and p not in sys.path:
            sys.path.insert(0, p)


def _build_nc():
    import concourse.bass as bass
    import concourse.mybir as mybir
    from concourse.masks import make_identity
    from concourse.tile import TileContext

    f32 = mybir.dt.float32
    bf16 = mybir.dt.bfloat16
    AF = mybir.ActivationFunctionType

    nc = bass.Bass()
    x_d = nc.dram_tensor("input", [B_SH, E], f32, kind="ExternalInput")
    h_d = nc.dram_tensor("hidden_state", [B_SH, H], f32, kind="ExternalInput")
    w_d = {n: nc.dram_tensor(n, [E, H], f32, kind="ExternalInput") for n in W_NAMES}
    b_d = {n: nc.dram_tensor(n, [1, H], f32, kind="ExternalInput") for n in B_NAMES}
    out_d = nc.dram_tensor("output", [B_SH, H], bf16, kind="ExternalOutput")

    with TileContext(nc) as tc:
        with (
            tc.tile_pool(name="sb", bufs=1) as sb,
            tc.tile_pool(name="psum", bufs=1, space="PSUM") as pp,
        ):
            ident = sb.tile([P, P], bf16, tag="ident", bufs=1)
            make_identity(nc, ident[:])
            identf = sb.tile([P, P], f32, tag="identf", bufs=1)
            make_identity(nc, identf[:])

            # ~5us of throwaway matmuls at t=0: trips the PE HAM activity
            # window so the clock is at 2.4GHz by the time real work lands
            # (otherwise the whole DMA-paced startup runs at 1.2GHz).
            warm_ps = pp.tile([P, 4 * P], f32, tag="tr", bufs=2)
            for i in range(48):
                nc.tensor.matmul(
                    warm_ps[:, 0:P], ident[:], ident[:],
                    start=(i == 0), stop=(i == 47),
                )

            # persistent transposed activations: [P, chunk, batch] bf16
            xT = sb.tile([P, KE, B_SH], bf16, tag="xT", bufs=1)
            hT = sb.tile([P, KE, B_SH], bf16, tag="hT", bufs=1)
            rhT = sb.tile([P, KE, B_SH], bf16, tag="rhT", bufs=1)
            uT = sb.tile([P, KE, B_SH], bf16, tag="uT", bufs=1)
            cT = sb.tile([P, KE, B_SH], bf16, tag="cT", bufs=1)

            # biases land transposed: [feature%128, j]
            bias_t = {}
            for g, nm in (("u", "bu"), ("r", "br"), ("c", "bc")):
                bt = sb.tile([P, NJ], f32, tag=f"bias_{g}", bufs=1)
                for j in range(NJ):
                    nc.gpsimd.dma_start(
                        bt[:, j : j + 1],
                        b_d[nm][0:1, j * P : (j + 1) * P].rearrange("a p -> p a"),
                    )
                bias_t[g] = bt

            def load_nat_chunk(src_d, bi, pieces=1):
                """DMA one [128, E] f32 row-chunk."""
                nat = sb.tile([P, E], f32, tag="nat", bufs=5)
                w = E // pieces
                for q in range(pieces):
                    csl = slice(q * w, (q + 1) * w)
                    nc.sync.dma_start(nat[:, csl], src_d[bi * P : (bi + 1) * P, csl])
                return nat

            def transpose_chunk(nat, dstT, bi):
                """PE-transpose a [128, E] f32 chunk into bf16 dstT[:, :, bi*P...].
                Two half-bank f32 PSUM batches; the drain copy does the
                f32->bf16 conversion."""
                for q in range(2):
                    trp = pp.tile([P, 4 * P], f32, tag="tr", bufs=2)
                    for t in range(4):
                        e = 4 * q + t
                        nc.tensor.transpose(
                            trp[:, t * P : (t + 1) * P],
                            nat[:, e * P : (e + 1) * P],
                            identf[:],
                        )
                    nc.vector.tensor_copy(
                        dstT[:, 4 * q : 4 * q + 4, bi * P : (bi + 1) * P],
                        trp[:].rearrange("p (t c) -> p t c", t=4),
                    )

            w_sb = {}

            def get_wt(wname, k):
                if (wname, k) not in w_sb:
                    w_sb[(wname, k)] = sb.tile(
                        [P, E], bf16, tag="w", name=f"w_{wname}_{k}", bufs=32
                    )
                return w_sb[(wname, k)]

            def load_w_chunks(wname, ks, cols=None):
                ks = list(ks)
                if cols is None:
                    # full-width loads in k-pairs: one DMA trigger + two
                    # casts per pair (halves HWDGE trigger overhead)
                    for i in range(0, len(ks), 2):
                        k0, k1 = ks[i], ks[i + 1]
                        wt0, wt1 = get_wt(wname, k0), get_wt(wname, k1)
                        ws = sb.tile(
                            [P, 2, E], f32, tag="ws2",
                            name=f"ws_{wname}_{k0}p", bufs=2,
                        )
                        nc.sync.dma_start(
                            ws[:],
                            w_d[wname][k0 * P : (k1 + 1) * P, :].rearrange(
                                "(q p) c -> p q c", p=P
                            ),
                        )
                        nc.vector.tensor_copy(wt0[:], ws[:, 0, :])
                        nc.vector.tensor_copy(wt1[:], ws[:, 1, :])
                    return
                for k in ks:
                    wt = get_wt(wname, k)
                    n_cols = cols.stop - cols.start
                    ws = sb.tile(
                        [P, n_cols],
                        f32,
                        tag="ws",
                        name=f"ws_{wname}_{k}_{cols.start}",
                        bufs=3,
                    )
                    nc.sync.dma_start(ws[:], w_d[wname][k * P : (k + 1) * P, cols])
                    nc.vector.tensor_copy(wt[:, cols], ws[:])

            # matmul phases for one gate, in weight-arrival order
            def gate_phases(wx, wh, rhsT):
                return (
                    (wx, xT, range(KE)),
                    (wh, rhsT, range(KE)),
                )

            def kouter_waves(phases, jgroup, n, interleave=None):
                """k-outer matmuls for a group of j's sharing PSUM residency.
                `interleave` maps phase index -> callback emitted before that
                phase's matmuls (used to slot DMA/transpose work into the
                paced stream at its arrival position)."""
                nsl = slice(n * BN, (n + 1) * BN)
                ps = {
                    j: pp.tile([P, BN], f32, tag="mm", name=f"mm_{j}_{n}", bufs=6)
                    for j in jgroup
                }
                first_key = (0, phases[0][2][0])
                last_key = (len(phases) - 1, phases[-1][2][-1])
                for pi, (wname, src, ks) in enumerate(phases):
                    if interleave and pi in interleave:
                        interleave[pi]()
                    for k in ks:
                        for j in jgroup:
                            nc.tensor.matmul(
                                ps[j][:],
                                w_sb[(wname, k)][:, j * P : (j + 1) * P],
                                src[:, k, nsl],
                                start=(pi, k) == first_key,
                                stop=(pi, k) == last_key,
                            )
                return ps

            def jouter_tile(phases, j, n):
                """j-outer: one PSUM tile accumulating its full contraction."""
                nsl = slice(n * BN, (n + 1) * BN)
                ps = pp.tile([P, BN], f32, tag="mm", name=f"mm_{j}_{n}", bufs=6)
                first_key = (0, phases[0][2][0])
                last_key = (len(phases) - 1, phases[-1][2][-1])
                for pi, (wname, src, ks) in enumerate(phases):
                    for k in ks:
                        nc.tensor.matmul(
                            ps[:],
                            w_sb[(wname, k)][:, j * P : (j + 1) * P],
                            src[:, k, nsl],
                            start=(pi, k) == first_key,
                            stop=(pi, k) == last_key,
                        )
                return ps

            def finish_r(ps, j, n):
                nsl = slice(n * BN, (n + 1) * BN)
                nc.scalar.activation(
                    rhT[:, j, nsl], ps[:], AF.Sigmoid, bias=bias_t["r"][:, j : j + 1]
                )
                nc.vector.tensor_mul(rhT[:, j, nsl], rhT[:, j, nsl], hT[:, j, nsl])

            def finish_u(ps, j, n):
                nsl = slice(n * BN, (n + 1) * BN)
                nc.scalar.activation(
                    uT[:, j, nsl], ps[:], AF.Sigmoid, bias=bias_t["u"][:, j : j + 1]
                )

            def finish_c(ps, j, n):
                nsl = slice(n * BN, (n + 1) * BN)
                nc.scalar.activation(
                    cT[:, j, nsl], ps[:], AF.Tanh, bias=bias_t["c"][:, j : j + 1]
                )
                # c = h + u*(c' - h), in place in cT
                nc.vector.tensor_sub(cT[:, j, nsl], cT[:, j, nsl], hT[:, j, nsl])
                nc.vector.tensor_mul(cT[:, j, nsl], cT[:, j, nsl], uT[:, j, nsl])
                nc.vector.tensor_add(cT[:, j, nsl], cT[:, j, nsl], hT[:, j, nsl])

            # ---- phase 1 + gate r batch-half 0, paced by DMA arrival ----
            # DMA ring order: x b0-3, Wxr k0-7, x b4-7, h b0-3,
            #                 Whr[:, j0-5] k0-7, h b4-7, Whr[:, j6-7] k0-7.
            # PE packing: x03 transposes, x-part matmuls (j0-5), x47/h03
            # transposes (fill the Whr-left DMA window), h-part matmuls
            # (j0-5, paced by Whr-left), h47 transposes, j6-7 mini-wave.
            for bi in range(4):
                transpose_chunk(load_nat_chunk(x_d, bi, pieces=2 if bi == 0 else 1), xT, bi)
            load_w_chunks("Wxr", range(KE))

            def stage_a():
                for bi in range(4, 8):
                    transpose_chunk(load_nat_chunk(x_d, bi), xT, bi)
                for bi in range(4):
                    transpose_chunk(load_nat_chunk(h_d, bi), hT, bi)
                load_w_chunks("Whr", range(KE), cols=slice(0, 6 * P))

            r_phases = gate_phases("Wxr", "Whr", hT)
            ps = kouter_waves(
                r_phases, range(0, 6), 0, interleave={1: stage_a}
            )
            for j in range(0, 6):
                finish_r(ps[j], j, 0)
            # j6-7 mini-wave: x-part first (weights resident), then the h b4-7
            # transposes fill the Whr-right DMA window, then the h-part.
            psj = {
                j: pp.tile([P, BN], f32, tag="mm", name=f"mm_{j}_0", bufs=6)
                for j in range(6, 8)
            }
            for k in range(KE):
                for j in range(6, 8):
                    nc.tensor.matmul(
                        psj[j][:],
                        w_sb[("Wxr", k)][:, j * P : (j + 1) * P],
                        xT[:, k, 0:BN],
                        start=(k == 0), stop=False,
                    )
            for bi in range(4, 8):
                transpose_chunk(load_nat_chunk(h_d, bi), hT, bi)
            load_w_chunks("Whr", range(KE), cols=slice(6 * P, E))
            for k in range(KE):
                for j in range(6, 8):
                    nc.tensor.matmul(
                        psj[j][:],
                        w_sb[("Whr", k)][:, j * P : (j + 1) * P],
                        hT[:, k, 0:BN],
                        start=False, stop=(k == KE - 1),
                    )
            for j in range(6, 8):
                finish_r(psj[j], j, 0)
            # prefetch gate u weights behind gate r compute
            load_w_chunks("Wxu", range(KE))
            load_w_chunks("Whu", range(KE))
            # gate r, batch-half 1 (dense): j-outer
            for j in range(NJ):
                ps = jouter_tile(r_phases, j, 1)
                finish_r(ps, j, 1)

            # ---- gate u (dense); gate c weights prefetch mid-gate so their
            # casts never gate u's own matmul semaphores ----
            u_phases = gate_phases("Wxu", "Whu", hT)
            for n in range(NB):
                for j in range(NJ):
                    ps = jouter_tile(u_phases, j, n)
                    finish_u(ps, j, n)
                    if n == 0 and j == NJ - 1:
                        load_w_chunks("Wxc", range(KE))
                    if n == 1 and j == 0:
                        load_w_chunks("Whc", range(KE))

            # ---- gate c (dense) + blend + output fold ----
            # Output transposes for each j's finished batch-half run right
            # after its blend, overlapping later tiles' matmuls; the store
            # fires once both halves of ost_j are assembled.
            c_phases = gate_phases("Wxc", "Whc", rhT)
            ost_j = {}
            for n in range(NB):
                for j in range(NJ):
                    ps = jouter_tile(c_phases, j, n)
                    finish_c(ps, j, n)
                    if n == 0:
                        ost_j[j] = sb.tile(
                            [P, KE, P], bf16, tag="ost", name=f"ost_{j}", bufs=8
                        )
                    half = range(4 * n, 4 * n + 4)
                    trp = pp.tile([P, 4 * P], bf16, tag="tr", bufs=2)
                    for qi, bi in enumerate(half):
                        nc.tensor.transpose(
                            trp[:, qi * P : (qi + 1) * P],
                            cT[:, j, bi * P : (bi + 1) * P],
                            ident[:],
                        )
                    nc.vector.tensor_copy(
                        ost_j[j][:, 4 * n : 4 * n + 4, :],
                        trp[:].rearrange("p (b c) -> p b c", b=4),
                    )
                    if n == NB - 1:
                        nc.sync.dma_start(
                            out_d[:, j * P : (j + 1) * P].rearrange(
                                "(b p) c -> p b c", p=P
                            ),
                            ost_j[j][:],
                        )

    _split_matmul_waits(nc, mybir)
    return nc


def _split_matmul_waits(nc, mybir):
    """Walrus codegen allows only one sync-wait on a Matmult (it lowers to an
    LDW+MM pair).  Spill extra waits onto a PE NoOp placed just before."""
    n_fixed = 0
    blocks = list(nc.m.functions[0].blocks)
    origs = [list(b.instructions) for b in blocks]
    spill_nops = {}  # id(inst) -> [nop insts]
    for orig in origs:
        for inst in orig:
            si = inst.sync_info
            if si is not None and si.on_wait and len(si.on_wait) > 1:
                waits = list(si.on_wait)
                eng = nc.engines[inst.engine]
                nops = []
                for w in waits[:-1]:
                    nop = eng.nop(hint="waitspill").ins
                    nop.sync_info = mybir.SyncInfo(on_wait=[w], on_update=[])
                    nops.append(nop)
                inst.sync_info = mybir.SyncInfo(
                    on_wait=waits[-1:], on_update=list(si.on_update or [])
                )
                spill_nops[id(inst)] = nops
                n_fixed += 1
    for blk, orig in zip(blocks, origs):
        new_list = []
        for inst in orig:
            if id(inst) in spill_nops:
                new_list.extend(spill_nops[id(inst)])
            new_list.append(inst)
        # rebuilding from `orig` also drops any freshly created nops that
        # bass appended to this block's tail
        blk.instructions[:] = new_list
    return n_fixed


def get_nc():
    if "nc" not in _NC_CACHE:
        _ensure_paths()
        _NC_CACHE["nc"] = _build_nc()
    return _NC_CACHE["nc"]


def kernel(**inputs):
    _ensure_paths()
    from concourse.bass_utils import run_bass_kernel_spmd

    nc = get_nc()

    x = np.ascontiguousarray(np.asarray(inputs["input"], dtype=np.float32))
    h = np.ascontiguousarray(np.asarray(inputs["hidden_state"], dtype=np.float32))
    shared = {
        n: np.ascontiguousarray(np.asarray(inputs[n], dtype=np.float32))
        for n in W_NAMES + B_NAMES
    }
    in_maps = []
    for c in range(NCORES):
        m = {
            "input": x[c * B_SH : (c + 1) * B_SH],
            "hidden_state": h[c * B_SH : (c + 1) * B_SH],
        }
        m.update(shared)
        in_maps.append(m)

    res = run_bass_kernel_spmd(nc, in_maps, list(range(NCORES)))
    out = np.concatenate(
        [np.asarray(res.results[c]["output"]) for c in range(NCORES)], axis=0
    )
    return out.astype(np.float32)
